# revision 6
# baseline (speedup 1.0000x reference)
"""Single-head unscaled attention (B=8, T=2048, D=1024, NODES=1024) on 8 trn2 cores.

Sharding: data-parallel over batch — core b computes batch element b end-to-end.
Weights are replicated to every core.

Math trick: S = Q K^T = X Wq (X Wk)^T = X M X^T with M = Wq Wk^T computed once
(1024^3 MACs) instead of K = X Wk (2048*1024^2 MACs) — saves ~12% of PE work.

Per-core pipeline (all matmuls fp16 in / fp32 PSUM accumulate):
  prologue:
    Wq^T, Wk^T   via PE transpose of casted loads          [n, d]
    M    = Wq Wk^T        (lhsT=Wq^T, rhs=Wk^T)            [d, e]
    X^T  via PE transpose                                  [d, t]
    A^T  = M^T X^T        (lhsT=M,    rhs=X^T)             [e, t]   (A = X M)
    V    = X Wv           (lhsT=X^T,  rhs=Wv)              [t, n]
  per q-tile (128 rows), software-pipelined by one stage:
    S    = A^T.T X^T      -> PSUM [128, 2048]
    softmax: block-max (DVE) -> exp+row-sum fused on ACT -> P fp16
    P^T  via PE transpose (2 groups of 8, batched copies)
    O    = P^T.T V        -> PSUM [128, 1024]; O *= 1/rowsum; DMA out
  PT(q)/O(q) are emitted after S(q+1) so the PE never waits on softmax.
"""

from contextlib import ExitStack

import numpy as np

import concourse.bass as bass
import concourse.mybir as mybir
import concourse.tile as tile
from concourse import bacc
from concourse.bass import ts
from concourse.masks import make_identity

P = 128
T = 2048
D = 1024
NO = 1024
B = 8
TT = T // P   # 16 tiles of 128 along t
DT = D // P   # 8 tiles along d (and along e)
NT = NO // P  # 8 tiles along nodes

F16 = mybir.dt.float16
F32 = mybir.dt.float32
AX = mybir.AxisListType
EXP = mybir.ActivationFunctionType.Exp


def _attention_body(tc, out, x, wq, wk, wv):
    nc = tc.nc
    x3 = x.rearrange("(t p) d -> t p d", p=P)
    o3 = out.rearrange("(t p) n -> t p n", p=P)

    with ExitStack() as ctx:
        const = ctx.enter_context(tc.tile_pool(name="const", bufs=1))
        persist = ctx.enter_context(tc.tile_pool(name="persist", bufs=1))
        # 1-bank psum accumulators/transpose staging, shared prologue+attention
        prps = ctx.enter_context(tc.tile_pool(name="prps", bufs=2, space="PSUM"))

        ident = const.tile([P, P], F16, tag="ident")
        make_identity(nc, ident)

        xt = persist.tile([P, DT, T], F16, tag="xt")     # X^T [d, t]
        at = persist.tile([P, DT, T], F16, tag="at")     # A^T [e, t]
        v = persist.tile([P, TT, NO], F16, tag="v")      # V   [t, n]
        m16 = persist.tile([P, DT, NO], F16, tag="m16")  # M   [d, e]
        wv16 = persist.tile([P, DT, NO], F16, tag="wv16")

        # ---------------- prologue ----------------
        with tc.tile_pool(name="wt", bufs=1) as wtp, tc.tile_pool(
            name="stage", bufs=6
        ) as stage, tc.tile_pool(name="tpsum", bufs=3, space="PSUM") as tpsum:
            wqT = wtp.tile([P, NT, D], F16, tag="wqT")   # Wq^T [n, d]
            wkT = wtp.tile([P, NT, D], F16, tag="wkT")

            # Wq, Wk first (M needs only these): load f32 in 512-col chunks
            # (faster first-byte), cast (GPSIMD), PE-transpose in 4-block
            # groups, batched copy per group alternating DVE/ACT
            cp_flip = [0]

            def group_copy(dst, src):
                if cp_flip[0] % 2 == 0:
                    nc.vector.tensor_copy(dst, src)
                else:
                    nc.scalar.copy(dst, src)
                cp_flip[0] += 1

            for wap, dstT in ((wq, wqT), (wk, wkT)):
                w3 = wap.rearrange("(do p) n -> do p n", p=P)
                for do in range(DT):
                    for h in range(2):
                        ws = stage.tile([P, 512], F32, tag="ws")
                        nc.sync.dma_start(ws, w3[do][:, ts(h, 512)])
                        wh = stage.tile([P, 512], F16, tag="wh")
                        nc.gpsimd.tensor_copy(wh, ws)
                        tp = tpsum.tile([P, 4, P], F16, tag="tp")
                        for no in range(4):
                            nc.tensor.transpose(tp[:, no], wh[:, ts(no, P)], ident)
                        group_copy(dstT[:, ts(h, 4), ts(do, P)], tp)

            # M[d, e] = sum_n Wq^T[n, d] Wk^T[n, e]
            for dt_ in range(DT):
                for eb in range(2):
                    ps = prps.tile([P, 512], F32, tag="ps")
                    for nt in range(NT):
                        nc.tensor.matmul(
                            ps,
                            wqT[:, nt, ts(dt_, P)],
                            wkT[:, nt, ts(eb, 512)],
                            start=(nt == 0),
                            stop=(nt == NT - 1),
                        )
                    nc.vector.tensor_copy(m16[:, dt_, ts(eb, 512)], ps)

            # X: load, cast, transpose (DMAs stream behind the W loads)
            for t_ in range(TT):
                for h in range(2):
                    xs = stage.tile([P, 512], F32, tag="ws")
                    nc.sync.dma_start(xs, x3[t_][:, ts(h, 512)])
                    xh = stage.tile([P, 512], F16, tag="wh")
                    nc.gpsimd.tensor_copy(xh, xs)
                    tp = tpsum.tile([P, 4, P], F16, tag="tp")
                    for do in range(4):
                        nc.tensor.transpose(tp[:, do], xh[:, ts(do, P)], ident)
                    group_copy(xt[:, ts(h, 4), ts(t_, P)], tp)

            # A^T[e, q] = sum_d M[d, e] X^T[d, q]
            for et in range(DT):
                for qb in range(4):
                    ps = prps.tile([P, 512], F32, tag="ps")
                    for dt_ in range(DT):
                        nc.tensor.matmul(
                            ps,
                            m16[:, dt_, ts(et, P)],
                            xt[:, dt_, ts(qb, 512)],
                            start=(dt_ == 0),
                            stop=(dt_ == DT - 1),
                        )
                    nc.vector.tensor_copy(at[:, et, ts(qb, 512)], ps)

            # Wv last (V is the last prologue consumer)
            wv3 = wv.rearrange("(do p) n -> do p n", p=P)
            for do in range(DT):
                ws = stage.tile([P, NO], F32, tag="ws")
                nc.sync.dma_start(ws, wv3[do])
                nc.gpsimd.tensor_copy(wv16[:, do, :], ws)

        def v_chunk(t_):
            for nb in range(2):
                ps = prps.tile([P, 512], F32, tag="ps")
                for dt_ in range(DT):
                    nc.tensor.matmul(
                        ps,
                        xt[:, dt_, ts(t_, P)],
                        wv16[:, dt_, ts(nb, 512)],
                        start=(dt_ == 0),
                        stop=(dt_ == DT - 1),
                    )
                nc.vector.tensor_copy(v[:, t_, ts(nb, 512)], ps)

        # all but the last two V chunks; those fill the first softmax gap
        for t_ in range(TT - 2):
            v_chunk(t_)

        # ---------------- attention ----------------
        with tc.tile_pool(name="spsum", bufs=1, space="PSUM") as spsum, tc.tile_pool(
            name="opsum", bufs=1, space="PSUM"
        ) as opsum, tc.tile_pool(name="soft", bufs=2) as soft, tc.tile_pool(
            name="ptp", bufs=2
        ) as ptp, tc.tile_pool(name="outp", bufs=2) as outp:

            def emit_pt_o(p16_, bsum_, q_):
                # P^T via PE transpose: 2 groups of 8, one batched copy each
                ptt = ptp.tile([P, 2, NT * P], F16, tag="ptt")
                for c in range(2):
                    pt_ps = prps.tile([P, NT, P], F16, tag="ps")
                    for j in range(NT):
                        nc.tensor.transpose(
                            pt_ps[:, j], p16_[:, ts(c * NT + j, P)], ident
                        )
                    nc.vector.tensor_copy(ptt[:, c], pt_ps)
                o_ps = opsum.tile([P, 2, 512], F32, tag="o")
                for c in range(2):
                    for j in range(NT):
                        k_ = c * NT + j
                        for nb in range(2):
                            nc.tensor.matmul(
                                o_ps[:, nb],
                                ptt[:, c, ts(j, P)],
                                v[:, k_, ts(nb, 512)],
                                start=(k_ == 0),
                                stop=(k_ == TT - 1),
                            )
                rsum = soft.tile([P, 1], F32, tag="rsum")
                nc.vector.tensor_reduce(rsum, bsum_, axis=AX.X, op=mybir.AluOpType.add)
                inv = soft.tile([P, 1], F32, tag="inv")
                nc.vector.reciprocal(inv, rsum)
                ob = outp.tile([P, NO], F32, tag="ob")
                for nb in range(2):
                    nc.vector.tensor_scalar_mul(ob[:, ts(nb, 512)], o_ps[:, nb], inv)
                nc.sync.dma_start(o3[q_], ob)

            prev = None
            for q_ in range(TT):
                s = spsum.tile([P, 4, 512], F32, tag="s")
                for et in range(DT):
                    for kb in range(4):
                        nc.tensor.matmul(
                            s[:, kb],
                            at[:, et, ts(q_, P)],
                            xt[:, et, ts(kb, 512)],
                            start=(et == 0),
                            stop=(et == DT - 1),
                        )
                if q_ == 0:
                    v_chunk(TT - 2)
                    v_chunk(TT - 1)
                # PT/O of the previous tile first: unblocks the PE before the
                # softmax reduces queue up on DVE
                if prev is not None:
                    emit_pt_o(*prev)
                bmax = soft.tile([P, 4], F32, tag="bmax")
                for kb in range(4):
                    nc.vector.tensor_reduce(
                        bmax[:, kb : kb + 1], s[:, kb], axis=AX.X, op=mybir.AluOpType.max
                    )
                negmax = soft.tile([P, 1], F32, tag="negmax")
                nc.vector.tensor_reduce(
                    negmax, bmax, axis=AX.X, op=mybir.AluOpType.max, negate=True
                )
                p16 = soft.tile([P, T], F16, tag="p16")
                bsum = soft.tile([P, 4], F32, tag="bsum")
                for kb in range(4):
                    nc.scalar.activation(
                        p16[:, ts(kb, 512)],
                        s[:, kb],
                        EXP,
                        bias=negmax,
                        scale=1.0,
                        accum_out=bsum[:, kb : kb + 1],
                    )
                prev = (p16, bsum, q_)
            emit_pt_o(*prev)


_CACHED_NC = {}


def _build(iters=1):
    global _CACHED_NC
    if iters in _CACHED_NC:
        return _CACHED_NC[iters]
    nc = bacc.Bacc("TRN2", target_bir_lowering=False, debug=False, num_devices=1)
    x = nc.dram_tensor("x", (T, D), F32, kind="ExternalInput").ap()
    wq = nc.dram_tensor("wq", (D, NO), F32, kind="ExternalInput").ap()
    wk = nc.dram_tensor("wk", (D, NO), F32, kind="ExternalInput").ap()
    wv = nc.dram_tensor("wv", (D, NO), F32, kind="ExternalInput").ap()
    out = nc.dram_tensor("out", (T, NO), F32, kind="ExternalOutput").ap()
    with tile.TileContext(nc) as tc:
        for _ in range(iters):
            _attention_body(tc, out, x, wq, wk, wv)
    nc.compile()
    _CACHED_NC[iters] = nc
    return nc


def kernel(inputs, Wq, Wk, Wv, trace=False):
    from concourse.bass_utils import run_bass_kernel_spmd

    nc = _build()
    inputs = np.ascontiguousarray(inputs, dtype=np.float32)
    Wq = np.ascontiguousarray(Wq, dtype=np.float32)
    Wk = np.ascontiguousarray(Wk, dtype=np.float32)
    Wv = np.ascontiguousarray(Wv, dtype=np.float32)
    in_maps = [
        {"x": inputs[b], "wq": Wq, "wk": Wk, "wv": Wv} for b in range(B)
    ]
    res = run_bass_kernel_spmd(nc, in_maps, core_ids=list(range(B)), trace=False)
    out = np.stack([r["out"] for r in res.results], axis=0)
    return out


# revision 7
# speedup vs baseline: 2.1136x; 2.1136x over previous
"""Single-head unscaled attention (B=8, T=2048, D=1024, NODES=1024) on 8 trn2 cores.

Sharding: data-parallel over batch — core b computes batch element b end-to-end.
Weights are replicated to every core.

Math trick: S = Q K^T = X Wq (X Wk)^T = X M X^T with M = Wq Wk^T computed once
(1024^3 MACs) instead of K = X Wk (2048*1024^2 MACs) — saves ~12% of PE work.

Per-core pipeline (all matmuls fp16 in / fp32 PSUM accumulate, 1024-col moving
operands where PSUM allows — fewer, fatter PE instructions):
  prologue:
    Wq^T, Wk^T   via PE transpose of casted loads          [n, d]
    M    = Wq Wk^T        (lhsT=Wq^T, rhs=Wk^T)            [d, e]
    X^T  via PE transpose                                  [d, t]
    A^T  = M^T X^T        (lhsT=M,    rhs=X^T)             [e, t]   (A = X M)
    V    = X Wv           (lhsT=X^T,  rhs=Wv)              [t, n]
  per q-tile (128 rows), software-pipelined by one stage:
    S    = A^T.T X^T      -> PSUM [128, 2048]
    softmax: block-max (DVE) -> exp+row-sum fused on ACT -> P fp16
    P^T  via PE transpose (4 groups of 4, batched copies)
    O    = P^T.T V        -> PSUM [128, 1024]; O *= 1/rowsum; DMA out
  PT(q)/O(q) are emitted after S(q+1) so the PE never waits on softmax.
"""

from contextlib import ExitStack

import numpy as np

import concourse.bass as bass
import concourse.mybir as mybir
import concourse.tile as tile
from concourse import bacc
from concourse.bass import ts
from concourse.masks import make_identity

P = 128
T = 2048
D = 1024
NO = 1024
B = 8
TT = T // P   # 16 tiles of 128 along t
DT = D // P   # 8 tiles along d (and along e)
NT = NO // P  # 8 tiles along nodes

F16 = mybir.dt.float16
F32 = mybir.dt.float32
AX = mybir.AxisListType
EXP = mybir.ActivationFunctionType.Exp


def _attention_body(tc, out, x, wq, wk, wv):
    nc = tc.nc
    x3 = x.rearrange("(t p) d -> t p d", p=P)
    o3 = out.rearrange("(t p) n -> t p n", p=P)

    with ExitStack() as ctx:
        const = ctx.enter_context(tc.tile_pool(name="const", bufs=1))
        persist = ctx.enter_context(tc.tile_pool(name="persist", bufs=1))
        # 1-bank psum pool: M/V accumulators and P^T transpose staging
        prps = ctx.enter_context(tc.tile_pool(name="prps", bufs=2, space="PSUM"))

        ident = const.tile([P, P], F16, tag="ident")
        make_identity(nc, ident)

        xt = persist.tile([P, DT, T], F16, tag="xt")     # X^T [d, t]
        at = persist.tile([P, DT, T], F16, tag="at")     # A^T [e, t]
        v = persist.tile([P, TT, NO], F16, tag="v")      # V   [t, n]
        m16 = persist.tile([P, DT, NO], F16, tag="m16")  # M   [d, e]
        wv16 = persist.tile([P, DT, NO], F16, tag="wv16")

        # ---------------- prologue ----------------
        with tc.tile_pool(name="wt", bufs=1) as wtp, tc.tile_pool(
            name="stage", bufs=6
        ) as stage, tc.tile_pool(name="tpsum", bufs=2, space="PSUM") as tpsum, tc.tile_pool(
            name="accps", bufs=2, space="PSUM"
        ) as accps:
            wqT = wtp.tile([P, NT, D], F16, tag="wqT")   # Wq^T [n, d]
            wkT = wtp.tile([P, NT, D], F16, tag="wkT")

            # Wq, Wk first (M needs only these): load f32 in 512-col chunks
            # (faster first-byte), cast (GPSIMD), PE-transpose in 4-block
            # groups, batched copy per group alternating DVE/ACT
            cp_flip = [0]

            def group_copy(dst, src):
                if cp_flip[0] % 2 == 0:
                    nc.vector.tensor_copy(dst, src)
                else:
                    nc.scalar.copy(dst, src)
                cp_flip[0] += 1

            for wap, dstT in ((wq, wqT), (wk, wkT)):
                w3 = wap.rearrange("(do p) n -> do p n", p=P)
                for do in range(DT):
                    for h in range(2):
                        ws = stage.tile([P, 512], F32, tag="ws")
                        nc.sync.dma_start(ws, w3[do][:, ts(h, 512)])
                        wh = stage.tile([P, 512], F16, tag="wh")
                        nc.gpsimd.tensor_copy(wh, ws)
                        tp = tpsum.tile([P, 4, P], F16, tag="tp")
                        for no in range(4):
                            nc.tensor.transpose(tp[:, no], wh[:, ts(no, P)], ident)
                        group_copy(dstT[:, ts(h, 4), ts(do, P)], tp)

            # M[d, e] = sum_n Wq^T[n, d] Wk^T[n, e]  (1024-col rhs, 2-bank psum)
            for dt_ in range(DT):
                ps = accps.tile([P, NO], F32, tag="ps")
                for nt in range(NT):
                    nc.tensor.matmul(
                        ps,
                        wqT[:, nt, ts(dt_, P)],
                        wkT[:, nt, :],
                        start=(nt == 0),
                        stop=(nt == NT - 1),
                    )
                nc.vector.tensor_copy(m16[:, dt_, :], ps)

            # X: load, cast, transpose (DMAs stream behind the W loads)
            for t_ in range(TT):
                for h in range(2):
                    xs = stage.tile([P, 512], F32, tag="ws")
                    nc.sync.dma_start(xs, x3[t_][:, ts(h, 512)])
                    xh = stage.tile([P, 512], F16, tag="wh")
                    nc.gpsimd.tensor_copy(xh, xs)
                    tp = tpsum.tile([P, 4, P], F16, tag="tp")
                    for do in range(4):
                        nc.tensor.transpose(tp[:, do], xh[:, ts(do, P)], ident)
                    group_copy(xt[:, ts(h, 4), ts(t_, P)], tp)

            # A^T[e, q] = sum_d M[d, e] X^T[d, q]  (1024-col rhs)
            for et in range(DT):
                for qb in range(2):
                    ps = accps.tile([P, NO], F32, tag="ps")
                    for dt_ in range(DT):
                        nc.tensor.matmul(
                            ps,
                            m16[:, dt_, ts(et, P)],
                            xt[:, dt_, ts(qb, NO)],
                            start=(dt_ == 0),
                            stop=(dt_ == DT - 1),
                        )
                    nc.vector.tensor_copy(at[:, et, ts(qb, NO)], ps)

            # Wv last (V is the last prologue consumer)
            wv3 = wv.rearrange("(do p) n -> do p n", p=P)
            for do in range(DT):
                ws = stage.tile([P, NO], F32, tag="ws2")
                nc.sync.dma_start(ws, wv3[do])
                nc.gpsimd.tensor_copy(wv16[:, do, :], ws)

        def v_chunk(t_):
            for nb in range(2):
                ps = prps.tile([P, 512], F32, tag="ps")
                for dt_ in range(DT):
                    nc.tensor.matmul(
                        ps,
                        xt[:, dt_, ts(t_, P)],
                        wv16[:, dt_, ts(nb, 512)],
                        start=(dt_ == 0),
                        stop=(dt_ == DT - 1),
                    )
                nc.vector.tensor_copy(v[:, t_, ts(nb, 512)], ps)

        # all but the last two V chunks; those fill the first softmax gap
        for t_ in range(TT - 2):
            v_chunk(t_)

        # ---------------- attention ----------------
        with tc.tile_pool(name="spsum", bufs=1, space="PSUM") as spsum, tc.tile_pool(
            name="opsum", bufs=1, space="PSUM"
        ) as opsum, tc.tile_pool(name="soft", bufs=2) as soft, tc.tile_pool(
            name="ptp", bufs=2
        ) as ptp, tc.tile_pool(name="outp", bufs=2) as outp:

            def emit_pt_o(p16_, bsum_, q_):
                # P^T via PE transpose: 4 groups of 4, one batched copy each
                ptt = ptp.tile([P, 4, 4 * P], F16, tag="ptt")
                for g in range(4):
                    pt_ps = prps.tile([P, 4, P], F16, tag="ps")
                    for j in range(4):
                        nc.tensor.transpose(
                            pt_ps[:, j], p16_[:, ts(4 * g + j, P)], ident
                        )
                    nc.vector.tensor_copy(ptt[:, g], pt_ps)
                o_ps = opsum.tile([P, NO], F32, tag="o")
                for g in range(4):
                    for j in range(4):
                        k_ = 4 * g + j
                        nc.tensor.matmul(
                            o_ps,
                            ptt[:, g, ts(j, P)],
                            v[:, k_, :],
                            start=(k_ == 0),
                            stop=(k_ == TT - 1),
                        )
                rsum = soft.tile([P, 1], F32, tag="rsum")
                nc.vector.tensor_reduce(rsum, bsum_, axis=AX.X, op=mybir.AluOpType.add)
                inv = soft.tile([P, 1], F32, tag="inv")
                nc.vector.reciprocal(inv, rsum)
                ob = outp.tile([P, NO], F32, tag="ob")
                nc.vector.tensor_scalar_mul(ob, o_ps, inv)
                nc.sync.dma_start(o3[q_], ob)

            prev = None
            for q_ in range(TT):
                s = spsum.tile([P, 2, NO], F32, tag="s")
                for et in range(DT):
                    for kb in range(2):
                        nc.tensor.matmul(
                            s[:, kb],
                            at[:, et, ts(q_, P)],
                            xt[:, et, ts(kb, NO)],
                            start=(et == 0),
                            stop=(et == DT - 1),
                        )
                if q_ == 0:
                    v_chunk(TT - 2)
                    v_chunk(TT - 1)
                # PT/O of the previous tile first: unblocks the PE before the
                # softmax reduces queue up on DVE
                if prev is not None:
                    emit_pt_o(*prev)
                bmax = soft.tile([P, 2], F32, tag="bmax")
                for kb in range(2):
                    nc.vector.tensor_reduce(
                        bmax[:, kb : kb + 1], s[:, kb], axis=AX.X, op=mybir.AluOpType.max
                    )
                negmax = soft.tile([P, 1], F32, tag="negmax")
                nc.vector.tensor_reduce(
                    negmax, bmax, axis=AX.X, op=mybir.AluOpType.max, negate=True
                )
                p16 = soft.tile([P, T], F16, tag="p16")
                bsum = soft.tile([P, 2], F32, tag="bsum")
                for kb in range(2):
                    nc.scalar.activation(
                        p16[:, ts(kb, NO)],
                        s[:, kb],
                        EXP,
                        bias=negmax,
                        scale=1.0,
                        accum_out=bsum[:, kb : kb + 1],
                    )
                prev = (p16, bsum, q_)
            emit_pt_o(*prev)


_CACHED_NC = {}


def _build(iters=1):
    global _CACHED_NC
    if iters in _CACHED_NC:
        return _CACHED_NC[iters]
    nc = bacc.Bacc("TRN2", target_bir_lowering=False, debug=False, num_devices=1)
    x = nc.dram_tensor("x", (T, D), F32, kind="ExternalInput").ap()
    wq = nc.dram_tensor("wq", (D, NO), F32, kind="ExternalInput").ap()
    wk = nc.dram_tensor("wk", (D, NO), F32, kind="ExternalInput").ap()
    wv = nc.dram_tensor("wv", (D, NO), F32, kind="ExternalInput").ap()
    out = nc.dram_tensor("out", (T, NO), F32, kind="ExternalOutput").ap()
    with tile.TileContext(nc) as tc:
        for _ in range(iters):
            _attention_body(tc, out, x, wq, wk, wv)
    nc.compile()
    _CACHED_NC[iters] = nc
    return nc


def kernel(inputs, Wq, Wk, Wv, trace=False):
    from concourse.bass_utils import run_bass_kernel_spmd

    nc = _build()
    inputs = np.ascontiguousarray(inputs, dtype=np.float32)
    Wq = np.ascontiguousarray(Wq, dtype=np.float32)
    Wk = np.ascontiguousarray(Wk, dtype=np.float32)
    Wv = np.ascontiguousarray(Wv, dtype=np.float32)
    in_maps = [
        {"x": inputs[b], "wq": Wq, "wk": Wk, "wv": Wv} for b in range(B)
    ]
    res = run_bass_kernel_spmd(nc, in_maps, core_ids=list(range(B)), trace=False)
    out = np.stack([r["out"] for r in res.results], axis=0)
    return out
